# revision 13
# baseline (speedup 1.0000x reference)
"""Trainium2 Bass kernel for DebiasSoftConLoss (SupCon-style loss with
confidence-weighted mask), 8-way row-sharded.

Math (B=4096, V=2, D=128, N=V*B=8192, T=0.07):
  C = cat(unbind(features,1))        # [N, D], L2-normalized rows
  loss_i = (log(denom_i + eps)*s2_i - s1_i) / (s2_i or 1);  out = mean loss_i
  denom_i = sum_{j != i} exp((dot_ij - dot_ii)/T)
  s1/s2 collapse to per-class sums (O(N*D)) and are computed on the host in
  f64; the device computes ONLY the O(N^2) part: the dot matrix row-block and
  the exp row sums.

Device strategy per core (1024 rows x 8192 cols):
  - fp8(e4m3) matmuls (no perf mode -> Fast Weight Load): C is scaled by 32
    (keeps values in fp8 normal range) and laid out [128(d), cols].  PSUM
    accumulates 1024*dot in f32.  One 512-col f32 matmul per PSUM bank.
  - Per-core column ROTATION by the core's row offset puts each core's
    diagonal in column-group 0, so one SPMD program works for all cores, and
    lhsT (anchor rows) is just the first 1024 columns of the rotated ct tile.
  - exp row sums are split across engines:
      ACT tiles (20): table-exp from PSUM with per-row bias -dot_ii/T and
        accum_out row sums (exact; diagonal tiles always in group m=0 so the
        self term is exp(0)=1, subtracted on the host).
      DVE tiles (12): Schraudolph trick -- i16 = round(psum*A + B_row) is
        the bf16 BIT PATTERN of exp(arg) with a piecewise-linear mantissa
        (+-4% per element, mean-centered via CORR); tensor_scalar writes
        int16.  Row sums: a same-row PAIR of approx-exp tiles is added
        elementwise on GpSimd (bf16), then one DVE reduce covers both.
  - Host: fp8 quantization, dii = |row|^2 in the same fp8 arithmetic as the
    PE diagonal, class sums g_c/S_c, s1/s2, final log + mean in f64.
"""

import numpy as np

B = 4096
V = 2
D = 128
N = B * V
CORES = 8
RPC = N // CORES          # rows per core = 1024
RT = RPC // 128           # row tiles per core = 8
GW = 1024                 # column group width (2 PSUM banks -> 4 slots)
MG = N // GW              # column groups per row tile = 8
TEMP = 0.07
INVT = 1.0 / TEMP
EPS = 1e-9
SC = 32.0                 # fp8 scale; PSUM holds SC^2 * dot
SC2 = SC * SC
L2E = 1.4426950408889634
# mean of (1+f)/2^f over f~U[0,1): PWL overestimate; center it (in lsb of
# the 7-bit bf16 mantissa).
CORR = 7.364
A16 = 128.0 * L2E * INVT / SC2   # i16 = psum*A16 + b16_row

# Per row tile t: the tuple of engines for column groups m=0..7.
# 'A' = ACT exact exp; 'V' = DVE Schraudolph.  m=0 must be 'A' (diagonal).
# Same-row 'V' tiles are summed on GpSimd (pairs, then pairs of pairs);
# one DVE reduce covers each combined tile.
PAT_4V = ('A', 'V', 'A', 'V', 'A', 'V', 'A', 'V')
PAT_3V = ('A', 'V', 'A', 'V', 'A', 'V', 'A', 'A')
ROW_PAT = [PAT_4V, PAT_3V, PAT_4V, PAT_3V, PAT_4V, PAT_3V, PAT_4V, PAT_3V]

_CACHE = {}


def _build_program():
    import concourse.tile as tile
    from concourse import bacc, mybir
    from concourse.bass import ds, ts

    f32 = mybir.dt.float32
    fp8 = mybir.dt.float8e4
    i16 = mybir.dt.int16
    bf16 = mybir.dt.bfloat16
    AF = mybir.ActivationFunctionType
    OP = mybir.AluOpType

    nc = bacc.Bacc(None, target_bir_lowering=False)

    ct_d = nc.dram_tensor("ct", [128, N], fp8, kind="ExternalInput")
    negb_d = nc.dram_tensor("negb", [128, RT], f32, kind="ExternalInput")
    b16_d = nc.dram_tensor("b16", [128, RT], f32, kind="ExternalInput")
    rsum_d = nc.dram_tensor("rsum", [128, RT], f32, kind="ExternalOutput")

    with tile.TileContext(nc) as tc:
        with (
            tc.tile_pool(name="big", bufs=1) as big,
            tc.tile_pool(name="sm", bufs=1) as sm,
            tc.tile_pool(name="scr", bufs=4) as scr,
            tc.tile_pool(name="pair", bufs=4) as pairp,
            tc.tile_pool(name="ps", bufs=2, space="PSUM") as ps,
        ):
            # ---- input DMAs: first column group first so matmul 0 starts
            # early; biases are tiny and needed by the first consumers ----
            sb_ct = big.tile([128, N], fp8)
            nc.sync.dma_start(out=sb_ct[:, 0:512], in_=ct_d[:, 0:512])
            sb_negb = sm.tile([128, RT], f32)
            nc.sync.dma_start(out=sb_negb[:, :], in_=negb_d[:, :])
            sb_b16 = sm.tile([128, RT], f32)
            nc.sync.dma_start(out=sb_b16[:, :], in_=b16_d[:, :])
            nc.sync.dma_start(out=sb_ct[:, 512:2048], in_=ct_d[:, 512:2048])
            nc.sync.dma_start(out=sb_ct[:, 2048:4096], in_=ct_d[:, 2048:4096])
            nc.sync.dma_start(out=sb_ct[:, 4096:6144], in_=ct_d[:, 4096:6144])
            nc.sync.dma_start(out=sb_ct[:, 6144:8192], in_=ct_d[:, 6144:8192])

            dsum = sm.tile([128, RT, MG], f32)
            nc.gpsimd.memset(dsum[:, :, :], 0.0)

            for t in range(RT):
                lhsT = sb_ct[:, ts(t, 128)]
                vtiles = []          # bf16 APs of this row's approx-exp tiles
                vslot = None         # dsum slot for the row's combined V sum
                for m in range(MG):
                    pt = ps.tile([128, GW], f32, tag="ps")
                    for k in range(GW // 512):
                        nc.tensor.matmul(
                            pt[:, ts(k, 512)],
                            lhsT=lhsT,
                            rhs=sb_ct[:, ds(m * GW + k * 512, 512)],
                            start=True,
                            stop=True,
                        )
                    if ROW_PAT[t][m] == 'A':
                        nc.scalar.activation(
                            out=pt[:, :],
                            in_=pt[:, :],
                            func=AF.Exp,
                            bias=sb_negb[:, t : t + 1],
                            scale=INVT / SC2,
                            accum_out=dsum[:, t, m : m + 1],
                        )
                    else:
                        es = scr.tile([128, GW], i16, tag="es")
                        nc.vector.tensor_scalar(
                            es[:, :], pt[:, :], A16, sb_b16[:, t : t + 1],
                            OP.mult, OP.add,
                        )
                        if vslot is None:
                            vslot = m
                        vtiles.append(es[:, :].bitcast(bf16))
                        # left-fold on GpSimd as soon as two operands exist
                        if len(vtiles) == 2:
                            a = vtiles.pop(0)
                            b = vtiles.pop(0)
                            q = pairp.tile([128, GW], bf16, tag="pp")
                            nc.gpsimd.tensor_tensor(q[:, :], a, b, OP.add)
                            vtiles.append(q[:, :])
                # one DVE reduce per row over the combined V tile
                if vtiles:
                    assert len(vtiles) == 1
                    nc.vector.reduce_sum(
                        out=dsum[:, t, vslot : vslot + 1],
                        in_=vtiles[0],
                        axis=mybir.AxisListType.X,
                    )

            rsum_sb = sm.tile([128, RT], f32)
            nc.vector.reduce_sum(
                out=rsum_sb[:, :], in_=dsum[:, :, :], axis=mybir.AxisListType.X
            )
            nc.sync.dma_start(out=rsum_d[:, :], in_=rsum_sb[:, :])

    nc.compile()
    return nc


def _marshal(features, max_probs, labels):
    import ml_dtypes

    feats = np.ascontiguousarray(np.asarray(features, dtype=np.float32))
    C = np.ascontiguousarray(feats.transpose(1, 0, 2).reshape(N, D))

    Cq8 = (C * SC).astype(ml_dtypes.float8_e4m3)        # [N, D] fp8, scaled
    Cqf = Cq8.astype(np.float64)
    dii_s = np.sum(Cqf * Cqf, axis=1)                   # = SC2 * dot_ii, f64

    base = np.ascontiguousarray(Cq8.T)                  # [128(d), N]

    negb_full = (-dii_s * INVT / SC2).astype(np.float32)          # ACT bias
    b16_full = (128.0 * 127.0 - CORR - dii_s * (128.0 * L2E * INVT / SC2)
                ).astype(np.float32)                              # DVE bias

    in_maps = []
    for k in range(CORES):
        r0 = k * RPC
        ct = np.ascontiguousarray(np.roll(base, -r0, axis=1))
        negb = np.ascontiguousarray(
            negb_full[r0 : r0 + RPC].reshape(RT, 128).T)
        b16 = np.ascontiguousarray(
            b16_full[r0 : r0 + RPC].reshape(RT, 128).T)
        in_maps.append({"ct": ct, "negb": negb, "b16": b16})
    return in_maps, C, dii_s


def _host_loss(rsums, C, dii_s, max_probs, labels):
    """rsums: [CORES, 128, RT] device row sums (incl. the self term ~1)."""
    mp = np.tile(np.asarray(max_probs, np.float64).reshape(B), V)
    lab = np.tile(np.asarray(labels).reshape(B), V)

    Cd = C.astype(np.float64)
    ncls = int(lab.max()) + 1
    S_c = np.zeros(ncls)
    g = np.zeros((ncls, D))
    for c in range(ncls):
        sel = lab == c
        S_c[c] = mp[sel].sum()
        g[c] = (mp[sel, None] * Cd[sel]).sum(0)
    q = np.einsum("nd,nd->n", Cd, g[lab])               # C_i . g_{lab_i}
    S_i = S_c[lab]
    dii = dii_s / SC2                                   # device diagonal
    s2 = mp * (S_i - mp)
    s1 = mp * (q - dii * S_i) * INVT
    # rsums[k, p, t] is the row sum of row k*RPC + t*128 + p
    rs = rsums.transpose(0, 2, 1).reshape(N).astype(np.float64)
    L = np.log(rs - 1.0 + EPS)
    s2p = np.where(s2 == 0, 1.0, s2)
    loss = (L * s2 - s1) / s2p
    return np.float32(loss.mean())


def _run_raw(in_maps, **kw):
    from concourse.bass_utils import run_bass_kernel_spmd

    if "nc" not in _CACHE:
        _CACHE["nc"] = _build_program()
    return run_bass_kernel_spmd(
        _CACHE["nc"], in_maps, core_ids=list(range(CORES)), **kw
    )


def kernel(features, max_probs, labels):
    in_maps, C, dii_s = _marshal(features, max_probs, labels)
    res = _run_raw(in_maps)
    rsums = np.stack([r["rsum"] for r in res.results])
    return _host_loss(rsums, C, dii_s, max_probs, labels)


# revision 14
# speedup vs baseline: 1.1242x; 1.1242x over previous
"""Trainium2 Bass kernel for DebiasSoftConLoss (SupCon-style loss with
confidence-weighted mask), 8-way row-sharded.

Math (B=4096, V=2, D=128, N=V*B=8192, T=0.07):
  C = cat(unbind(features,1))        # [N, D], L2-normalized rows
  loss_i = (log(denom_i + eps)*s2_i - s1_i) / (s2_i or 1);  out = mean loss_i
  denom_i = sum_{j != i} exp((dot_ij - dot_ii)/T)
  s1/s2 collapse to per-class sums (O(N*D)) and are computed on the host in
  f64; the device computes ONLY the O(N^2) part: the dot matrix row-block and
  the exp row sums.

Device strategy per core (1024 rows x 8192 cols):
  - fp8(e4m3) matmuls (no perf mode -> Fast Weight Load): C is scaled by 32
    (keeps values in fp8 normal range) and laid out [128(d), cols].  PSUM
    accumulates 1024*dot in f32.  One 512-col f32 matmul per PSUM bank.
  - Per-core column ROTATION by the core's row offset puts each core's
    diagonal in column-group 0, so one SPMD program works for all cores, and
    lhsT (anchor rows) is just the first 1024 columns of the rotated ct tile.
  - exp row sums are split across engines:
      ACT tiles (20): table-exp from PSUM with per-row bias -dot_ii/T and
        accum_out row sums (exact; diagonal tiles always in group m=0 so the
        self term is exp(0)=1, subtracted on the host).
      DVE tiles (12): Schraudolph trick -- i16 = round(psum*A + B_row) is
        the bf16 BIT PATTERN of exp(arg) with a piecewise-linear mantissa
        (+-4% per element, mean-centered via CORR); tensor_scalar writes
        int16.  Row sums: a same-row PAIR of approx-exp tiles is added
        elementwise on GpSimd (bf16), then one DVE reduce covers both.
  - Host: fp8 quantization, dii = |row|^2 in the same fp8 arithmetic as the
    PE diagonal, class sums g_c/S_c, s1/s2, final log + mean in f64.
"""

import numpy as np

B = 4096
V = 2
D = 128
N = B * V
CORES = 8
RPC = N // CORES          # rows per core = 1024
RT = RPC // 128           # row tiles per core = 8
GW = 2048                 # column group width
MG = N // GW              # column groups per row tile = 4
TEMP = 0.07
INVT = 1.0 / TEMP
EPS = 1e-9
SC = 32.0                 # fp8 scale; PSUM holds SC^2 * dot
SC2 = SC * SC
L2E = 1.4426950408889634
# mean of (1+f)/2^f over f~U[0,1): PWL overestimate; center it (in lsb of
# the 7-bit bf16 mantissa).
CORR = 7.364
A16 = 128.0 * L2E * INVT / SC2   # i16 = psum*A16 + b16_row

# Per row tile t: the tuple of engines for column groups m=0..3.
# 'A' = ACT exact exp; 'V' = DVE Schraudolph.  m=0 must be 'A' (diagonal).
# Two 'V' in one row tile are paired: GpSimd adds them, one DVE reduce.
PAT_AAAA = ('A', 'A', 'A', 'A')
PAT_AVAV = ('A', 'V', 'A', 'V')
ROW_PAT = [PAT_AVAV, PAT_AVAV, PAT_AVAV, PAT_AAAA,
           PAT_AVAV, PAT_AVAV, PAT_AVAV, PAT_AAAA]

_CACHE = {}


def _build_program():
    import concourse.tile as tile
    from concourse import bacc, mybir
    from concourse.bass import ds, ts

    f32 = mybir.dt.float32
    fp8 = mybir.dt.float8e4
    i16 = mybir.dt.int16
    bf16 = mybir.dt.bfloat16
    AF = mybir.ActivationFunctionType
    OP = mybir.AluOpType

    nc = bacc.Bacc(None, target_bir_lowering=False)

    ct_d = nc.dram_tensor("ct", [128, N], fp8, kind="ExternalInput")
    negb_d = nc.dram_tensor("negb", [128, RT], f32, kind="ExternalInput")
    b16_d = nc.dram_tensor("b16", [128, RT], f32, kind="ExternalInput")
    rsum_d = nc.dram_tensor("rsum", [128, RT], f32, kind="ExternalOutput")

    with tile.TileContext(nc) as tc:
        with (
            tc.tile_pool(name="big", bufs=1) as big,
            tc.tile_pool(name="sm", bufs=1) as sm,
            tc.tile_pool(name="scr", bufs=4) as scr,
            tc.tile_pool(name="pair", bufs=4) as pairp,
            tc.tile_pool(name="ps", bufs=2, space="PSUM") as ps,
        ):
            # ---- input DMAs: first column group first so matmul 0 starts
            # early; biases are tiny and needed by the first consumers ----
            sb_ct = big.tile([128, N], fp8)
            nc.sync.dma_start(out=sb_ct[:, 0:512], in_=ct_d[:, 0:512])
            sb_negb = sm.tile([128, RT], f32)
            nc.sync.dma_start(out=sb_negb[:, :], in_=negb_d[:, :])
            sb_b16 = sm.tile([128, RT], f32)
            nc.sync.dma_start(out=sb_b16[:, :], in_=b16_d[:, :])
            nc.sync.dma_start(out=sb_ct[:, 512:2048], in_=ct_d[:, 512:2048])
            nc.sync.dma_start(out=sb_ct[:, 2048:4096], in_=ct_d[:, 2048:4096])
            nc.sync.dma_start(out=sb_ct[:, 4096:6144], in_=ct_d[:, 4096:6144])
            nc.sync.dma_start(out=sb_ct[:, 6144:8192], in_=ct_d[:, 6144:8192])

            dsum = sm.tile([128, RT, MG], f32)
            nc.gpsimd.memset(dsum[:, :, :], 0.0)

            # pair-sum tiles (GpSimd adds) whose DVE reduce is DEFERRED into
            # the all-ACT rows, where the DVE queue is naturally idle -- a
            # reduce between two tensor_scalars stalls the PSUM slot pipeline
            # and lets the PE clock-gate down.
            deferred = []

            def flush_deferred():
                for ap_, tt_, mm_ in deferred:
                    nc.vector.reduce_sum(
                        out=dsum[:, tt_, mm_ : mm_ + 1],
                        in_=ap_,
                        axis=mybir.AxisListType.X,
                    )
                deferred.clear()

            for t in range(RT):
                lhsT = sb_ct[:, ts(t, 128)]
                pend = None          # pending V tile awaiting its pair
                if ROW_PAT[t] == PAT_AAAA:
                    flush_deferred()
                for m in range(MG):
                    pt = ps.tile([128, GW], f32, tag="ps")
                    for k in range(GW // 512):
                        nc.tensor.matmul(
                            pt[:, ts(k, 512)],
                            lhsT=lhsT,
                            rhs=sb_ct[:, ds(m * GW + k * 512, 512)],
                            start=True,
                            stop=True,
                        )
                    if ROW_PAT[t][m] == 'A':
                        aout = scr.tile([128, GW], bf16, tag="aout")
                        nc.scalar.activation(
                            out=aout[:, :],
                            in_=pt[:, :],
                            func=AF.Exp,
                            bias=sb_negb[:, t : t + 1],
                            scale=INVT / SC2,
                            accum_out=dsum[:, t, m : m + 1],
                        )
                    else:
                        es = scr.tile([128, GW], i16, tag="es")
                        nc.vector.tensor_scalar(
                            es[:, :], pt[:, :], A16, sb_b16[:, t : t + 1],
                            OP.mult, OP.add,
                        )
                        if pend is None:
                            pend = (es, m)
                        else:
                            es0, m0 = pend
                            pend = None
                            psum2 = pairp.tile([128, GW], bf16, tag="pp")
                            nc.gpsimd.tensor_tensor(
                                psum2[:, :], es0[:, :].bitcast(bf16),
                                es[:, :].bitcast(bf16), OP.add,
                            )
                            deferred.append((psum2[:, :], t, m))
                if pend is not None:
                    es0, m0 = pend
                    deferred.append((es0[:, :].bitcast(bf16), t, m0))
            flush_deferred()

            rsum_sb = sm.tile([128, RT], f32)
            nc.vector.reduce_sum(
                out=rsum_sb[:, :], in_=dsum[:, :, :], axis=mybir.AxisListType.X
            )
            nc.sync.dma_start(out=rsum_d[:, :], in_=rsum_sb[:, :])

    nc.compile()
    return nc


def _marshal(features, max_probs, labels):
    import ml_dtypes

    feats = np.ascontiguousarray(np.asarray(features, dtype=np.float32))
    C = np.ascontiguousarray(feats.transpose(1, 0, 2).reshape(N, D))

    Cq8 = (C * SC).astype(ml_dtypes.float8_e4m3)        # [N, D] fp8, scaled
    Cqf = Cq8.astype(np.float64)
    dii_s = np.sum(Cqf * Cqf, axis=1)                   # = SC2 * dot_ii, f64

    base = np.ascontiguousarray(Cq8.T)                  # [128(d), N]

    negb_full = (-dii_s * INVT / SC2).astype(np.float32)          # ACT bias
    b16_full = (128.0 * 127.0 - CORR - dii_s * (128.0 * L2E * INVT / SC2)
                ).astype(np.float32)                              # DVE bias

    in_maps = []
    for k in range(CORES):
        r0 = k * RPC
        ct = np.ascontiguousarray(np.roll(base, -r0, axis=1))
        negb = np.ascontiguousarray(
            negb_full[r0 : r0 + RPC].reshape(RT, 128).T)
        b16 = np.ascontiguousarray(
            b16_full[r0 : r0 + RPC].reshape(RT, 128).T)
        in_maps.append({"ct": ct, "negb": negb, "b16": b16})
    return in_maps, C, dii_s


def _host_loss(rsums, C, dii_s, max_probs, labels):
    """rsums: [CORES, 128, RT] device row sums (incl. the self term ~1)."""
    mp = np.tile(np.asarray(max_probs, np.float64).reshape(B), V)
    lab = np.tile(np.asarray(labels).reshape(B), V)

    Cd = C.astype(np.float64)
    ncls = int(lab.max()) + 1
    S_c = np.zeros(ncls)
    g = np.zeros((ncls, D))
    for c in range(ncls):
        sel = lab == c
        S_c[c] = mp[sel].sum()
        g[c] = (mp[sel, None] * Cd[sel]).sum(0)
    q = np.einsum("nd,nd->n", Cd, g[lab])               # C_i . g_{lab_i}
    S_i = S_c[lab]
    dii = dii_s / SC2                                   # device diagonal
    s2 = mp * (S_i - mp)
    s1 = mp * (q - dii * S_i) * INVT
    # rsums[k, p, t] is the row sum of row k*RPC + t*128 + p
    rs = rsums.transpose(0, 2, 1).reshape(N).astype(np.float64)
    L = np.log(rs - 1.0 + EPS)
    s2p = np.where(s2 == 0, 1.0, s2)
    loss = (L * s2 - s1) / s2p
    return np.float32(loss.mean())


def _run_raw(in_maps, **kw):
    from concourse.bass_utils import run_bass_kernel_spmd

    if "nc" not in _CACHE:
        _CACHE["nc"] = _build_program()
    return run_bass_kernel_spmd(
        _CACHE["nc"], in_maps, core_ids=list(range(CORES)), **kw
    )


def kernel(features, max_probs, labels):
    in_maps, C, dii_s = _marshal(features, max_probs, labels)
    res = _run_raw(in_maps)
    rsums = np.stack([r["rsum"] for r in res.results])
    return _host_loss(rsums, C, dii_s, max_probs, labels)
